# revision 5
# baseline (speedup 1.0000x reference)
"""2D Haar DWT (analysis) on 8 Trainium2 NeuronCores — fp16 I/O,
DMA-engine-aware load balancing.

Input  x: (16, 64, 256, 256) f32  -> 1024 independent 256x256 images.
Output: tuple (LL, LH, HL, HH), each (16, 64, 128, 128) f32.

With Haar filters the DWT is a 2x2 butterfly: per 2x2 block (a b / c d),
with the 0.5 scale folded into a host-side prescale:
    se=a+c de=a-c so=b+d do=b-d ; LL=se+so LH=se-so HL=de+do HH=de-do
No matmul — 8 elementwise ops per chunk, 7 on VectorE (fp16 = DVE 2x
perf mode) + 1 on GpSimd to keep DVE under the DMA roofline.

Why fp16: the kernel is HBM/DMA-port bound. f32 moves 67MB/core
(~158us at the ~425GB/s port rate); fp16 halves that. Quantization
costs ~4e-4 l2 rel err (gate is 2e-2).

Why ragged partitions: SDMA engine 15 (serving SBUF partitions 92-95
and 124-127) suffers episodic external contention (~23GB/s vs 26.8).
The work unit is a "row-pair item" (2 rows x 256 cols of one image =
512 elems); items are freely redistributable across partitions since
the butterfly is elementwise per partition. Fast partitions (120) get
130 items, port-15 partitions (8) get 98, so engine 15 moves ~75% of
the bytes of the others. If the contention disappears this costs
<1.5us; while it persists it saves ~10us.

Chunks: 9 per core (8 ramp-up + 7x16 + 10 tail items on fast
partitions), every DMA contiguous or >=3KB-run strided.
"""

import numpy as np

import concourse.bacc as bacc
import concourse.tile as tile
from concourse import mybir
from concourse.bass_utils import run_bass_kernel_spmd

N_CORES = 8
B, C, H, W = 16, 64, 256, 256
N_IMG = B * C                    # 1024
P = N_IMG // N_CORES             # 128 images per core = partition dim
Wh = W // 2                      # 128
ITEMS = P * (H // 2)             # 16384 row-pair items per core
IW = 512                         # elems per item (2 rows x 256 cols)

# partition classes: port 15 serves partitions 92-95 and 124-127
N_FAST, N_SLOW = 120, 8
N_F, N_S = 130, 98               # items per fast / slow partition
assert N_FAST * N_F + N_SLOW * N_S == ITEMS
CF = [8, 16, 16, 16, 16, 16, 16, 16, 10]   # fast items per chunk
CS = [6, 12, 12, 12, 12, 12, 12, 12, 8]    # slow items per chunk
assert sum(CF) == N_F and sum(CS) == N_S and len(CF) == len(CS)
NCH = len(CF)
FOFF = np.cumsum([0] + CF).tolist()
SOFF = np.cumsum([0] + CS).tolist()
GP_OPS = 1                       # band ops offloaded to GpSimd (0 or 1)
XP_BUFS = 5
F16 = mybir.dt.float16

_CACHE = {}


def _butterfly(nc, xt, mid, op, cf):
    """Butterfly for one chunk. xt is [128, 4*cf*128] laid out
    [quad(a,c,b,d), item, w]; returns ot [128, 4*cf*128] = [band, item, w].
    Slow partitions compute garbage beyond their valid prefix; it is
    simply never stored."""
    q = cf * Wh
    a, c, b, d = (xt[:, j * q:(j + 1) * q] for j in range(4))
    se = mid.tile([P, q], F16, tag="se")
    de = mid.tile([P, q], F16, tag="de")
    so = mid.tile([P, q], F16, tag="so")
    do = mid.tile([P, q], F16, tag="do")
    nc.vector.tensor_add(se, a, c)
    nc.vector.tensor_sub(de, a, c)
    nc.vector.tensor_add(so, b, d)
    nc.vector.tensor_sub(do, b, d)
    ot = op.tile([P, 4 * q], F16, tag="ot")
    nc.vector.tensor_add(ot[:, 0 * q:1 * q], se, so)      # LL
    nc.vector.tensor_sub(ot[:, 1 * q:2 * q], se, so)      # LH
    nc.vector.tensor_add(ot[:, 2 * q:3 * q], de, do)      # HL
    if GP_OPS:
        nc.gpsimd.tensor_sub(ot[:, 3 * q:4 * q], de, do)  # HH
    else:
        nc.vector.tensor_sub(ot[:, 3 * q:4 * q], de, do)
    return ot


def _build_program():
    nc = bacc.Bacc(
        "TRN2",
        target_bir_lowering=False,
        debug=False,
        enable_asserts=False,
        num_devices=N_CORES,
    )
    xf1 = nc.dram_tensor("xf1", [92, N_F * IW], F16, kind="ExternalInput").ap()
    xf2 = nc.dram_tensor("xf2", [28, N_F * IW], F16, kind="ExternalInput").ap()
    xs1 = nc.dram_tensor("xs1", [4, N_S * IW], F16, kind="ExternalInput").ap()
    xs2 = nc.dram_tensor("xs2", [4, N_S * IW], F16, kind="ExternalInput").ap()
    of1 = nc.dram_tensor("of1", [92, N_F * IW], F16, kind="ExternalOutput").ap()
    of2 = nc.dram_tensor("of2", [28, N_F * IW], F16, kind="ExternalOutput").ap()
    os1 = nc.dram_tensor("os1", [4, N_S * IW], F16, kind="ExternalOutput").ap()
    os2 = nc.dram_tensor("os2", [4, N_S * IW], F16, kind="ExternalOutput").ap()

    with tile.TileContext(nc) as tc:
        with (
            tc.tile_pool(name="xp", bufs=XP_BUFS) as xp,
            tc.tile_pool(name="mid", bufs=3) as mid,
            tc.tile_pool(name="op", bufs=4) as op,
        ):
            for k in range(NCH):
                cf, cs = CF[k], CS[k]
                fc = slice(FOFF[k] * IW, (FOFF[k] + cf) * IW)
                sc = slice(SOFF[k] * IW, (SOFF[k] + cs) * IW)
                xt = xp.tile([P, 4 * cf * Wh], F16, tag="xt")
                nc.sync.dma_start(out=xt[0:92, :], in_=xf1[:, fc])
                nc.sync.dma_start(out=xt[96:124, :], in_=xf2[:, fc])
                # slow partitions: valid prefix of each quadrant only
                xtv = xt.rearrange("p (q t w) -> p q t w", q=4, t=cf, w=Wh)
                nc.sync.dma_start(
                    out=xtv[92:96, :, 0:cs, :],
                    in_=xs1[:, sc].rearrange("p (q t w) -> p q t w",
                                             q=4, t=cs, w=Wh))
                nc.sync.dma_start(
                    out=xtv[124:128, :, 0:cs, :],
                    in_=xs2[:, sc].rearrange("p (q t w) -> p q t w",
                                             q=4, t=cs, w=Wh))
                ot = _butterfly(nc, xt, mid, op, cf)
                otv = ot.rearrange("p (b t w) -> p b t w", b=4, t=cf, w=Wh)
                nc.scalar.dma_start(out=of1[:, fc], in_=ot[0:92, :])
                nc.scalar.dma_start(out=of2[:, fc], in_=ot[96:124, :])
                nc.scalar.dma_start(
                    out=os1[:, sc].rearrange("p (b t w) -> p b t w",
                                             b=4, t=cs, w=Wh),
                    in_=otv[92:96, :, 0:cs, :])
                nc.scalar.dma_start(
                    out=os2[:, sc].rearrange("p (b t w) -> p b t w",
                                             b=4, t=cs, w=Wh),
                    in_=otv[124:128, :, 0:cs, :])
    nc.compile()
    return nc


def _pack_core(quad):
    """quad: [ITEMS, 4, Wh] fp16 for one core (item j = img*128 + i).
    Returns the 4 DRAM input arrays."""
    fast = quad[:N_FAST * N_F].reshape(N_FAST, N_F, 4, Wh)
    slow = quad[N_FAST * N_F:].reshape(N_SLOW, N_S, 4, Wh)
    fcols = [fast[:, FOFF[k]:FOFF[k + 1]].transpose(0, 2, 1, 3)
             .reshape(N_FAST, CF[k] * IW) for k in range(NCH)]
    scols = [slow[:, SOFF[k]:SOFF[k + 1]].transpose(0, 2, 1, 3)
             .reshape(N_SLOW, CS[k] * IW) for k in range(NCH)]
    XF = np.concatenate(fcols, axis=1)
    XS = np.concatenate(scols, axis=1)
    return {
        "xf1": np.ascontiguousarray(XF[:92]),
        "xf2": np.ascontiguousarray(XF[92:]),
        "xs1": np.ascontiguousarray(XS[:4]),
        "xs2": np.ascontiguousarray(XS[4:]),
    }


def _unpack_core(r):
    """Inverse of _pack_core for the outputs: returns [4, ITEMS, Wh]."""
    OF = np.concatenate([r["of1"], r["of2"]], axis=0)     # [120, N_F*IW]
    OS = np.concatenate([r["os1"], r["os2"]], axis=0)     # [8, N_S*IW]
    fblk = [OF[:, FOFF[k] * IW:FOFF[k + 1] * IW]
            .reshape(N_FAST, 4, CF[k], Wh) for k in range(NCH)]
    sblk = [OS[:, SOFF[k] * IW:SOFF[k + 1] * IW]
            .reshape(N_SLOW, 4, CS[k], Wh) for k in range(NCH)]
    fast = np.concatenate(fblk, axis=2)                   # [120, 4, N_F, Wh]
    slow = np.concatenate(sblk, axis=2)                   # [8, 4, N_S, Wh]
    out = np.empty((4, ITEMS, Wh), dtype=np.float16)
    out[:, :N_FAST * N_F] = fast.transpose(1, 0, 2, 3).reshape(4, -1, Wh)
    out[:, N_FAST * N_F:] = slow.transpose(1, 0, 2, 3).reshape(4, -1, Wh)
    return out


def kernel(x, m_l0, m_l1, m_h0, m_h1):
    x = np.asarray(x, dtype=np.float32)
    assert x.shape == (B, C, H, W), x.shape

    if "nc" not in _CACHE:
        _CACHE["nc"] = _build_program()
    nc = _CACHE["nc"]

    # prescale by 0.5 (exact), quantize to fp16, split into row-pair
    # items of quadrants [a, c, b, d]
    x16 = (x.reshape(N_IMG, H, W) * np.float32(0.5)).astype(np.float16)
    # [n, i, f, w, e] -> [n, i, e, f, w] -> items
    xq = x16.reshape(N_IMG, H // 2, 2, Wh, 2).transpose(0, 1, 4, 2, 3)
    in_maps = []
    for s in range(N_CORES):
        quad = xq[s * P:(s + 1) * P].reshape(ITEMS, 4, Wh)
        in_maps.append(_pack_core(quad))

    res = run_bass_kernel_spmd(nc, in_maps, core_ids=list(range(N_CORES)))

    outs = []
    for s in range(N_CORES):
        o = _unpack_core(res.results[s])                  # [4, ITEMS, Wh]
        outs.append(o.reshape(4, P, H // 2, Wh))
    full = np.stack(outs, axis=1).reshape(4, B, C, H // 2, Wh)
    full = full.astype(np.float32)
    return (np.ascontiguousarray(full[0]), np.ascontiguousarray(full[1]),
            np.ascontiguousarray(full[2]), np.ascontiguousarray(full[3]))


# revision 6
# speedup vs baseline: 2.3484x; 2.3484x over previous
"""2D Haar DWT (analysis) on 8 Trainium2 NeuronCores — fp16 I/O.

Input  x: (16, 64, 256, 256) f32  -> 1024 independent 256x256 images.
Output: tuple (LL, LH, HL, HH), each (16, 64, 128, 128) f32.

With Haar filters the DWT is a 2x2 butterfly: per 2x2 block (a b / c d),
with the 0.5 scale folded into a host-side prescale:
    se=a+c de=a-c so=b+d do=b-d ; LL=se+so LH=se-so HL=de+do HH=de-do
No matmul — 8 elementwise ops per chunk, 7 on VectorE (fp16 = DVE 2x
perf mode) + 1 on GpSimd so DVE stays under the DMA roofline.

Why fp16: the kernel is HBM/DMA-port bound. f32 moves 67MB/core
(~158us at the ~425GB/s port rate); fp16 halves that (~80us floor).
Quantization costs ~4e-4 l2 rel err (gate is 2e-2).

Layout: host prescales by 0.5, quantizes to fp16, and rearranges each
chunk of row-pair items as [quad(a,c,b,d), item, w] per partition
(partition = image), so every vector op is a flat unit-stride slice
(DVE 2x mode requires step=1, 4B-aligned). Every DMA covers all 128
partitions with contiguous >=1KB/partition runs — partial-partition
DMAs measured catastrophically unbalanced across SDMA engines (4/16
engines get ~all bytes), so they are avoided entirely.

Chunks: first/last small (8 items) to shorten pipeline ramp and drain,
middle chunks 16 items (2MB transfers, 16KB/partition runs).
"""

import numpy as np

import concourse.bacc as bacc
import concourse.tile as tile
from concourse import mybir
from concourse.bass_utils import run_bass_kernel_spmd

N_CORES = 8
B, C, H, W = 16, 64, 256, 256
N_IMG = B * C                    # 1024
P = N_IMG // N_CORES             # 128 images per core = partition dim
Wh = W // 2                      # 128
NI = H // 2                      # 128 row-pair items per partition
IW = 512                         # elems per item (2 rows x 256 cols)

CF = [8, 16, 16, 16, 16, 16, 16, 16, 8]   # items per chunk
assert sum(CF) == NI
NCH = len(CF)
FOFF = np.cumsum([0] + CF).tolist()
GP_OPS = 1                       # band ops offloaded to GpSimd (0 or 1)
XP_BUFS = 5
F16 = mybir.dt.float16

_CACHE = {}


def _butterfly(nc, xt, mid, op, cf):
    """Butterfly for one chunk. xt is [128, 4*cf*128] laid out
    [quad(a,c,b,d), item, w]; returns ot [128, 4*cf*128] = [band, item, w]."""
    q = cf * Wh
    a, c, b, d = (xt[:, j * q:(j + 1) * q] for j in range(4))
    se = mid.tile([P, q], F16, tag="se")
    de = mid.tile([P, q], F16, tag="de")
    so = mid.tile([P, q], F16, tag="so")
    do = mid.tile([P, q], F16, tag="do")
    nc.vector.tensor_add(se, a, c)
    nc.vector.tensor_sub(de, a, c)
    nc.vector.tensor_add(so, b, d)
    nc.vector.tensor_sub(do, b, d)
    ot = op.tile([P, 4 * q], F16, tag="ot")
    nc.vector.tensor_add(ot[:, 0 * q:1 * q], se, so)      # LL
    nc.vector.tensor_sub(ot[:, 1 * q:2 * q], se, so)      # LH
    nc.vector.tensor_add(ot[:, 2 * q:3 * q], de, do)      # HL
    if GP_OPS:
        nc.gpsimd.tensor_sub(ot[:, 3 * q:4 * q], de, do)  # HH
    else:
        nc.vector.tensor_sub(ot[:, 3 * q:4 * q], de, do)
    return ot


def _build_program():
    nc = bacc.Bacc(
        "TRN2",
        target_bir_lowering=False,
        debug=False,
        enable_asserts=False,
        num_devices=N_CORES,
    )
    xb = nc.dram_tensor("xb", [P, NI * IW], F16, kind="ExternalInput").ap()
    ob = nc.dram_tensor("ob", [P, NI * IW], F16, kind="ExternalOutput").ap()

    with tile.TileContext(nc) as tc:
        with (
            tc.tile_pool(name="xp", bufs=XP_BUFS) as xp,
            tc.tile_pool(name="mid", bufs=3) as mid,
            tc.tile_pool(name="op", bufs=4) as op,
        ):
            for k in range(NCH):
                cf = CF[k]
                col = slice(FOFF[k] * IW, (FOFF[k] + cf) * IW)
                xt = xp.tile([P, 4 * cf * Wh], F16, tag="xt")
                nc.sync.dma_start(out=xt, in_=xb[:, col])
                ot = _butterfly(nc, xt, mid, op, cf)
                nc.scalar.dma_start(out=ob[:, col], in_=ot)
    nc.compile()
    return nc


def kernel(x, m_l0, m_l1, m_h0, m_h1):
    x = np.asarray(x, dtype=np.float32)
    assert x.shape == (B, C, H, W), x.shape

    if "nc" not in _CACHE:
        _CACHE["nc"] = _build_program()
    nc = _CACHE["nc"]

    # prescale by 0.5 (exact), quantize to fp16, split into row-pair
    # items with quadrant order [a, c, b, d]:
    # [n, i, f, w, e] -> [n, i, e, f, w]
    x16 = (x.reshape(N_IMG, H, W) * np.float32(0.5)).astype(np.float16)
    xq = x16.reshape(N_IMG, NI, 2, Wh, 2).transpose(0, 1, 4, 2, 3)
    in_maps = []
    for s in range(N_CORES):
        quad = xq[s * P:(s + 1) * P].reshape(P, NI, 4, Wh)
        # per chunk, per partition: [quad, item, w]
        cols = [quad[:, FOFF[k]:FOFF[k + 1]].transpose(0, 2, 1, 3)
                .reshape(P, CF[k] * IW) for k in range(NCH)]
        in_maps.append({"xb": np.ascontiguousarray(np.concatenate(cols, axis=1))})

    res = run_bass_kernel_spmd(nc, in_maps, core_ids=list(range(N_CORES)))

    outs = []
    for s in range(N_CORES):
        ob = res.results[s]["ob"]
        blks = [ob[:, FOFF[k] * IW:FOFF[k + 1] * IW]
                .reshape(P, 4, CF[k], Wh) for k in range(NCH)]
        outs.append(np.concatenate(blks, axis=2))         # [P, 4, NI, Wh]
    full = np.stack(outs, axis=0).reshape(B, C, 4, H // 2, Wh)
    full = full.astype(np.float32)
    return (np.ascontiguousarray(full[:, :, 0]),
            np.ascontiguousarray(full[:, :, 1]),
            np.ascontiguousarray(full[:, :, 2]),
            np.ascontiguousarray(full[:, :, 3]))


# revision 9
# speedup vs baseline: 2.9142x; 1.2409x over previous
"""2D Haar DWT (analysis) on 8 Trainium2 NeuronCores — fp16 I/O with
DMA-engine load shaping.

Input  x: (16, 64, 256, 256) f32  -> 1024 independent 256x256 images.
Output: tuple (LL, LH, HL, HH), each (16, 64, 128, 128) f32.

With Haar filters the DWT is a 2x2 butterfly: per 2x2 block (a b / c d),
with the 0.5 scale folded into a host-side prescale:
    se=a+c de=a-c so=b+d do=b-d ; LL=se+so LH=se-so HL=de+do HH=de-do
8 flat fp16 VectorE ops per chunk (DVE 2x perf mode). No matmul.

Why fp16: the kernel is HBM/DMA-port bound. f32 moves 67MB/core
(~158us at the ~425GB/s port rate); fp16 halves the bytes AND halves
DVE time. Quantization costs ~4e-4 l2 rel err (gate is 2e-2).

Why the 3-way transfer split: SDMA engine 15 suffers episodic external
contention (~22GB/s vs 26.8 measured). HWDGE deals a transfer's
descriptors (1 per partition) to engines in contiguous runs of
ceil(N/16) starting at engine 0 (probed): a [0:128] transfer loads all
16 engines evenly, a [0:120] transfer loads engines 0-14 only, and a
16-run strided transfer puts one run on each engine. Splitting each
chunk as  T1=[0:128]x(block-u) + T2=[0:120]xu + T3=[120:128]xu(16 runs)
gives engine 15 ~83% of the bytes of the others, matching its degraded
rate. Costs <1.5us if the contention vanishes; saves ~10us while it
persists. Partition counts like 92/28/4 are catastrophically
unbalanced (measured: 4 engines get ~all bytes) — avoided.

Chunks of [4, 12, 20x5, 12] items (item = 2 rows x 256 cols): small
first chunk shortens the pipeline ramp, small last chunk the drain.
"""

import numpy as np

import concourse.bacc as bacc
import concourse.tile as tile
from concourse import mybir
from concourse.bass_utils import run_bass_kernel_spmd

N_CORES = 8
B, C, H, W = 16, 64, 256, 256
N_IMG = B * C                    # 1024
P = N_IMG // N_CORES             # 128 images per core = partition dim
Wh = W // 2                      # 128
NI = H // 2                      # 128 row-pair items per partition
IW = 512                         # elems per item (2 rows x 256 cols)

CF = [4, 12, 20, 20, 20, 20, 20, 12]      # items per chunk
assert sum(CF) == NI
NCH = len(CF)
FOFF = np.cumsum([0] + CF).tolist()
# elems per partition routed via T2/T3 (engine-15 relief); 0 = no split
CU = [0, 1024, 1792, 1792, 1792, 1792, 1792, 1024]
XP_BUFS = 4
F16 = mybir.dt.float16

_CACHE = {}


def _butterfly(nc, xt, mid, op, cf):
    """8 flat VectorE ops; xt is [128, 4*cf*128] laid out
    [quad(a,c,b,d), item, w]; returns ot = [band(LL,LH,HL,HH), item, w]."""
    q = cf * Wh
    a, c, b, d = (xt[:, j * q:(j + 1) * q] for j in range(4))
    se = mid.tile([P, q], F16, tag="se")
    de = mid.tile([P, q], F16, tag="de")
    so = mid.tile([P, q], F16, tag="so")
    do = mid.tile([P, q], F16, tag="do")
    nc.vector.tensor_add(se, a, c)
    nc.vector.tensor_sub(de, a, c)
    nc.vector.tensor_add(so, b, d)
    nc.vector.tensor_sub(do, b, d)
    ot = op.tile([P, 4 * q], F16, tag="ot")
    nc.vector.tensor_add(ot[:, 0 * q:1 * q], se, so)   # LL
    nc.vector.tensor_sub(ot[:, 1 * q:2 * q], se, so)   # LH
    nc.vector.tensor_add(ot[:, 2 * q:3 * q], de, do)   # HL
    nc.vector.tensor_sub(ot[:, 3 * q:4 * q], de, do)   # HH
    return ot


def _col_plans():
    """Per-chunk (a_off, a_len, b_off, u) column geometry and totals."""
    plans, aoff, boff, coff = [], 0, 0, 0
    for k in range(NCH):
        blk, u = 512 * CF[k], CU[k]
        plans.append((aoff, blk - u, boff, coff, u))
        aoff += blk - u
        boff += u
        coff += 8 * u
    return plans, aoff, boff, coff


def _build_program():
    nc = bacc.Bacc(
        "TRN2",
        target_bir_lowering=False,
        debug=False,
        enable_asserts=False,
        num_devices=N_CORES,
    )
    plans, atot, btot, ctot = _col_plans()
    xa = nc.dram_tensor("xa", [P, atot], F16, kind="ExternalInput").ap()
    xb2 = nc.dram_tensor("xb2", [120, btot], F16, kind="ExternalInput").ap()
    xc = nc.dram_tensor("xc", [ctot], F16, kind="ExternalInput").ap()
    oa = nc.dram_tensor("oa", [P, atot], F16, kind="ExternalOutput").ap()
    ob2 = nc.dram_tensor("ob2", [120, btot], F16, kind="ExternalOutput").ap()
    oc = nc.dram_tensor("oc", [ctot], F16, kind="ExternalOutput").ap()

    with tile.TileContext(nc) as tc:
        with (
            tc.tile_pool(name="xp", bufs=XP_BUFS) as xp,
            tc.tile_pool(name="mid", bufs=3) as mid,
            tc.tile_pool(name="op", bufs=3) as op,
        ):
            for k in range(NCH):
                cf = CF[k]
                ao, alen, bo, co, u = plans[k]
                blk = 512 * cf
                xt = xp.tile([P, blk], F16, tag="xt")
                nc.sync.dma_start(out=xt[:, 0:alen], in_=xa[:, ao:ao + alen])
                if u:
                    nc.sync.dma_start(
                        out=xt[0:120, alen:blk], in_=xb2[:, bo:bo + u])
                    # [120:128] remainder as 16 runs -> one per engine
                    nc.sync.dma_start(
                        out=xt[120:128, alen:blk].rearrange(
                            "p (j w) -> p j w", j=2),
                        in_=xc[co:co + 8 * u].rearrange(
                            "(j p w) -> p j w", j=2, p=8))
                ot = _butterfly(nc, xt, mid, op, cf)
                nc.scalar.dma_start(out=oa[:, ao:ao + alen], in_=ot[:, 0:alen])
                if u:
                    nc.scalar.dma_start(
                        out=ob2[:, bo:bo + u], in_=ot[0:120, alen:blk])
                    nc.scalar.dma_start(
                        out=oc[co:co + 8 * u].rearrange(
                            "(j p w) -> p j w", j=2, p=8),
                        in_=ot[120:128, alen:blk].rearrange(
                            "p (j w) -> p j w", j=2))
    nc.compile()
    return nc


def kernel(x, m_l0, m_l1, m_h0, m_h1):
    x = np.asarray(x, dtype=np.float32)
    assert x.shape == (B, C, H, W), x.shape

    if "nc" not in _CACHE:
        _CACHE["nc"] = _build_program()
    nc = _CACHE["nc"]

    plans, atot, btot, ctot = _col_plans()

    # prescale by 0.5 (exact), quantize to fp16, quadrant order [a,c,b,d]:
    # [n, i, f, w, e] -> [n, i, e, f, w]
    x16 = (x.reshape(N_IMG, H, W) * np.float32(0.5)).astype(np.float16)
    xq = x16.reshape(N_IMG, NI, 2, Wh, 2).transpose(0, 1, 4, 2, 3)
    in_maps = []
    for s in range(N_CORES):
        quad = xq[s * P:(s + 1) * P].reshape(P, NI, 4, Wh)
        xa = np.empty((P, atot), dtype=np.float16)
        xb2 = np.empty((120, btot), dtype=np.float16)
        xc = np.empty(ctot, dtype=np.float16)
        for k in range(NCH):
            ao, alen, bo, co, u = plans[k]
            blk = (quad[:, FOFF[k]:FOFF[k + 1]].transpose(0, 2, 1, 3)
                   .reshape(P, 512 * CF[k]))
            xa[:, ao:ao + alen] = blk[:, 0:alen]
            if u:
                xb2[:, bo:bo + u] = blk[0:120, alen:]
                xc[co:co + 8 * u] = (blk[120:128, alen:]
                                     .reshape(8, 2, u // 2)
                                     .transpose(1, 0, 2).ravel())
        in_maps.append({"xa": xa, "xb2": xb2, "xc": xc})

    res = run_bass_kernel_spmd(nc, in_maps, core_ids=list(range(N_CORES)))

    outs = []
    for s in range(N_CORES):
        r = res.results[s]
        blks = []
        for k in range(NCH):
            ao, alen, bo, co, u = plans[k]
            blk = np.empty((P, 512 * CF[k]), dtype=np.float16)
            blk[:, 0:alen] = r["oa"][:, ao:ao + alen]
            if u:
                blk[0:120, alen:] = r["ob2"][:, bo:bo + u]
                blk[120:128, alen:] = (r["oc"][co:co + 8 * u]
                                       .reshape(2, 8, u // 2)
                                       .transpose(1, 0, 2)
                                       .reshape(8, u))
            blks.append(blk.reshape(P, 4, CF[k], Wh))
        outs.append(np.concatenate(blks, axis=2))         # [P, 4, NI, Wh]
    full = np.stack(outs, axis=0).reshape(B, C, 4, H // 2, Wh)
    full = full.astype(np.float32)
    return (np.ascontiguousarray(full[:, :, 0]),
            np.ascontiguousarray(full[:, :, 1]),
            np.ascontiguousarray(full[:, :, 2]),
            np.ascontiguousarray(full[:, :, 3]))
